# revision 12
# baseline (speedup 1.0000x reference)
"""GCN layer kernel for Trainium2, 8 NeuronCores — single-launch version.

Math (identical to reference):
    deg = bincount(row);  d = 1/sqrt(deg)
    h   = x @ W.T + b
    out = d * segment_sum(d[col] * h[col], row) + d^2 * h

Rewritten as aggregate-then-transform (linear map commutes with segment sum):
    U[r]   = sum_{edges (r,c)} d_c * x_c + d_r * x_r
    cc[r]  = sum_{edges (r,c)} d_c + d_r
    out[r] = d_r * (U[r] @ W.T) + (cc[r] * d_r) * b

One SPMD launch (destinations sharded across the 8 cores, identical program,
per-core data):
  * edges sorted by (dest block of 128, source chunk of 25088); x rows (fp16,
    256B) gathered in bulk with gpsimd.dma_gather.  Gathered edge i lands at
    SBUF partition i%128, tile i//128.
  * per 128-edge tile, a selection matrix st[e, dest_local] = (iota==dl)*d_c
    (fp16) is built with one fused tensor_scalar (is_equal, mult) — carrying
    the per-source d_c scaling — and one PE fp16 matmul accumulates
    slab^T @ st into the block's f32 PSUM tile U^T[feat, dest].
  * self term: one matmul with rhs = ident * d_r (diagonal of d) which also
    clears the full PSUM tile.
  * per block: U^T is already [feat, dest], so no transpose: one 128x128
    matmul with W^T, then scale by d_r and add (cc*d_r)*b (cc precomputed on
    host along with all index/degree prep).
Slot padding uses source row 0 with dest_local = -1 and d_c = 0 (selection
column is all zero), so padded gathers are harmless; per-(block,chunk) tile
counts are the max over cores, keeping shapes static across the SPMD program.
"""

import numpy as np
import sys

sys.path.insert(0, "/opt/trn_rl_repo")

import concourse.bacc as bacc
import concourse.tile as tile
from concourse import mybir
from concourse.bass_utils import run_bass_kernel_spmd
from concourse.masks import make_identity

NCORES = 8
P = 128
CHUNK = 25088  # dma_gather idx is int16: source chunks must stay < 32768 rows
SB = 4  # dest blocks per superblock (gather granularity)
SLAB_BUFS = 2
F32 = mybir.dt.float32
F16 = mybir.dt.float16
I16 = mybir.dt.int16

_cache = {}
LAST = {}  # populated on each kernel() call (for profiling in test.py)


def _build(meta):
    """Gather + selection-matmul segment sum + per-block W matmul."""
    din = meta["din"]
    dout = meta["dout"]
    n_y = meta["n_y"]  # padded x rows (nchunk * CHUNK)
    nblk = meta["nblk"]
    sblocks = meta["sblocks"]  # list of lists of block ids
    sb_tiles = meta["sb_tiles"]  # per sb: total tiles
    sb_calls = meta["sb_calls"]  # per sb: list of (chunk, tile_off_in_sb, ntiles)
    blk_runs = meta["blk_runs"]  # per block: list of (tile_off_in_sb, ntiles)
    tile_base = meta["tile_base"]  # per sb: global tile offset
    win_lo = meta["win_lo"]
    win_w = meta["win_w"]
    ttot = meta["ttot"]

    nc = bacc.Bacc(
        "TRN2",
        target_bir_lowering=False,
        debug=False,
        enable_asserts=False,
        num_devices=NCORES,
    )
    x_t = nc.dram_tensor("x_t", [n_y, din], F16, kind="ExternalInput").ap()
    xs_t = nc.dram_tensor("xs_t", [nblk * P, din], F16, kind="ExternalInput").ap()
    idx_t = nc.dram_tensor("idx_t", [P, ttot * 8], I16, kind="ExternalInput").ap()
    dl_t = nc.dram_tensor("dl_t", [P, ttot], F32, kind="ExternalInput").ap()
    dc_t = nc.dram_tensor("dc_t", [P, ttot], F32, kind="ExternalInput").ap()
    dv_t = nc.dram_tensor("dv_t", [P, nblk], F32, kind="ExternalInput").ap()
    cb_t = nc.dram_tensor("cb_t", [P, nblk], F32, kind="ExternalInput").ap()
    wt_t = nc.dram_tensor("wt_t", [din, dout], F32, kind="ExternalInput").ap()
    brep_t = nc.dram_tensor("brep_t", [P, dout], F32, kind="ExternalInput").ap()
    out_t = nc.dram_tensor("out_t", [nblk * P, dout], F32, kind="ExternalOutput").ap()

    max_sb_tiles = max(sb_tiles)

    with tile.TileContext(nc) as tc:
        with (
            tc.tile_pool(name="const", bufs=1) as cpool,
            tc.tile_pool(name="slab", bufs=SLAB_BUFS) as gpool,
            tc.tile_pool(name="sel", bufs=6) as selpool,
            tc.tile_pool(name="work", bufs=3) as wpool,
            tc.tile_pool(name="psum", bufs=2, space="PSUM") as ppool,
        ):
            ident = cpool.tile([P, P], dtype=F16)
            make_identity(nc, ident[:])
            iota_i = cpool.tile([P, P], dtype=mybir.dt.int32)
            nc.gpsimd.iota(iota_i[:], pattern=[[1, P]], base=0, channel_multiplier=0)
            iota_f = cpool.tile([P, P], dtype=F16)
            nc.vector.tensor_copy(iota_f[:], iota_i[:])
            wt_sb = cpool.tile([din, dout], dtype=F32)
            nc.sync.dma_start(out=wt_sb[:], in_=wt_t[:, :])
            brep_sb = cpool.tile([P, dout], dtype=F32)
            nc.sync.dma_start(out=brep_sb[:], in_=brep_t[:, :])
            dv_sb = cpool.tile([P, nblk], dtype=F32)
            nc.sync.dma_start(out=dv_sb[:], in_=dv_t[:, :])
            cb_sb = cpool.tile([P, nblk], dtype=F32)
            nc.sync.dma_start(out=cb_sb[:], in_=cb_t[:, :])

            xs_v = xs_t.rearrange("(t p) f -> p t f", p=P)
            out_v = out_t.rearrange("(t p) f -> p t f", p=P)
            for sbi, blks in enumerate(sblocks):
                nt_sb = sb_tiles[sbi]
                tb = tile_base[sbi]
                nb = len(blks)
                idx_sb = wpool.tile([P, max_sb_tiles * 8], dtype=I16, tag="idx")
                nc.sync.dma_start(
                    out=idx_sb[:, 0 : nt_sb * 8],
                    in_=idx_t[:, tb * 8 : (tb + nt_sb) * 8],
                )
                dl_sb = wpool.tile([P, max_sb_tiles], dtype=F32, tag="dl")
                nc.sync.dma_start(out=dl_sb[:, 0:nt_sb], in_=dl_t[:, tb : tb + nt_sb])
                dc_sb = wpool.tile([P, max_sb_tiles], dtype=F32, tag="dc")
                nc.sync.dma_start(out=dc_sb[:, 0:nt_sb], in_=dc_t[:, tb : tb + nt_sb])
                xs_sb = wpool.tile([P, SB, din], dtype=F16, tag="xs")
                nc.sync.dma_start(
                    out=xs_sb[:, 0:nb, :], in_=xs_v[:, blks[0] : blks[0] + nb, :]
                )
                slab = gpool.tile([P, max_sb_tiles, din], dtype=F16, tag="slab")
                for (c, toff, nt) in sb_calls[sbi]:
                    ni = nt * P
                    nc.gpsimd.dma_gather(
                        out_ap=slab[:, toff : toff + nt, :],
                        in_ap=x_t[c * CHUNK : (c + 1) * CHUNK, :],
                        idxs_ap=idx_sb[:, toff * 8 : (toff + nt) * 8],
                        num_idxs=ni,
                        num_idxs_reg=ni,
                        elem_size=din,
                        single_packet=False,
                    )
                osb_sb = wpool.tile([P, SB, dout], dtype=F32, tag="osb")
                for j, b in enumerate(blks):
                    # PSUM tile holds U^T: [feat, dest_local]
                    ups = ppool.tile([P, P], dtype=F32, space="PSUM", tag="ups")
                    runs = blk_runs[b]
                    ntb = sum(r[1] for r in runs)
                    # self term first: rhs = diag(d_r); clears the whole tile
                    dd = selpool.tile([P, P], dtype=F16, tag="dd")
                    nc.vector.tensor_scalar(
                        out=dd[:],
                        in0=ident[:],
                        scalar1=dv_sb[:, b : b + 1],
                        scalar2=None,
                        op0=mybir.AluOpType.mult,
                    )
                    nc.tensor.matmul(
                        out=ups[:],
                        lhsT=xs_sb[:, j, :],
                        rhs=dd[:],
                        start=True,
                        stop=(ntb == 0),
                    )
                    ti = 0
                    for (toff, nt) in runs:
                        for k in range(nt):
                            t_sb = toff + k
                            t_g = tb + t_sb
                            lo = int(win_lo[t_g])
                            w = int(win_w[t_g])
                            st = selpool.tile([P, P], dtype=F16, tag="st")
                            nc.vector.tensor_scalar(
                                out=st[:, 0:w],
                                in0=iota_f[:, 0:w],
                                scalar1=dl_sb[:, t_sb : t_sb + 1],
                                scalar2=dc_sb[:, t_sb : t_sb + 1],
                                op0=mybir.AluOpType.is_equal,
                                op1=mybir.AluOpType.mult,
                            )
                            nc.tensor.matmul(
                                out=ups[:, lo : lo + w],
                                lhsT=slab[:, t_sb, :],
                                rhs=st[:, 0:w],
                                start=False,
                                stop=(ti == ntb - 1),
                            )
                            ti += 1
                    # U^T -> SBUF, then out = d_r * (U @ W^T) + (cc*d_r) * b
                    usb = wpool.tile([P, P], dtype=F32, tag="usb")
                    nc.scalar.activation(
                        usb[:], ups[:], mybir.ActivationFunctionType.Copy
                    )
                    o2 = ppool.tile([P, dout], dtype=F32, space="PSUM", tag="o2")
                    nc.tensor.matmul(
                        out=o2[:], lhsT=usb[:], rhs=wt_sb[:], start=True, stop=True
                    )
                    t1 = wpool.tile([P, dout], dtype=F32, tag="t1")
                    nc.scalar.activation(
                        t1[:],
                        brep_sb[:],
                        mybir.ActivationFunctionType.Copy,
                        scale=cb_sb[:, b : b + 1],
                    )
                    t2 = wpool.tile([P, dout], dtype=F32, tag="t2")
                    nc.scalar.activation(
                        t2[:],
                        o2[:],
                        mybir.ActivationFunctionType.Copy,
                        scale=dv_sb[:, b : b + 1],
                    )
                    nc.vector.tensor_tensor(
                        out=osb_sb[:, j, :],
                        in0=t2[:],
                        in1=t1[:],
                        op=mybir.AluOpType.add,
                    )
                nc.sync.dma_start(
                    out=out_v[:, blks[0] : blks[0] + nb, :], in_=osb_sb[:, 0:nb, :]
                )
    nc.compile()
    return nc


def _prep(x, edge_index, W, b):
    N, din = x.shape
    dout = W.shape[0]
    npc = N // NCORES
    nblk = (npc + P - 1) // P
    npc_pad = nblk * P
    nchunk = (N + CHUNK - 1) // CHUNK
    n_y = nchunk * CHUNK

    row = np.asarray(edge_index[0], dtype=np.int64)
    col = np.asarray(edge_index[1], dtype=np.int64)
    deg = np.bincount(row, minlength=N)  # int
    d64 = 1.0 / np.sqrt(deg.astype(np.float64))
    d32 = d64.astype(np.float32)
    # cc[r] = sum_{edges (r,c)} d_c + d_r   (f64 accumulate on host)
    cc = np.bincount(row, weights=d64[col], minlength=N) + d64
    cbv = (cc * d64).astype(np.float32)  # coefficient of b per node

    order_e = np.argsort(row, kind="stable")
    row_s = row[order_e]
    col_s = col[order_e]
    rowstart = np.zeros(N + 1, dtype=np.int64)
    np.cumsum(deg, out=rowstart[1:])

    # ---- per-core edge lists (dest-sharded) --------------------------------
    # per core arrays: dest_local(0..npc_pad), col (global), sorted by
    # (block, chunk) with CSR (dest-ascending) order preserved inside.
    core_dl = []
    core_col = []
    counts = np.zeros((NCORES, nblk, nchunk), dtype=np.int64)
    for m in range(NCORES):
        lo, hi = rowstart[m * npc], rowstart[(m + 1) * npc]
        dl = row_s[lo:hi] - m * npc
        cl = col_s[lo:hi]
        blk = dl >> 7
        ch = cl // CHUNK
        o = np.lexsort((ch, blk))
        dl, cl, blk, ch = dl[o], cl[o], blk[o], ch[o]
        core_dl.append(dl)
        core_col.append(cl)
        np.add.at(counts[m], (blk, ch), 1)

    tcnt = (np.max(counts, axis=0) + P - 1) // P  # [nblk, nchunk] tiles
    # ---- static tile schedule ----------------------------------------------
    sblocks = [list(range(s, min(s + SB, nblk))) for s in range(0, nblk, SB)]
    sb_calls = []
    blk_runs = [None] * nblk
    sb_tiles = []
    tile_base = []
    tpos = {}  # (b, c) -> global tile offset
    gt = 0
    for sbi, blks in enumerate(sblocks):
        tile_base.append(gt)
        calls = []
        toff = 0
        for c in range(nchunk):
            nt = int(sum(tcnt[b, c] for b in blks))
            if nt:
                calls.append((c, toff, nt))
            for b in blks:
                if tcnt[b, c]:
                    tpos[(b, c)] = gt + toff
                    toff += int(tcnt[b, c])
        sb_calls.append(calls)
        for b in blks:
            blk_runs[b] = [
                (tpos[(b, c)] - gt, int(tcnt[b, c]))
                for c in range(nchunk)
                if tcnt[b, c]
            ]
        sb_tiles.append(toff)
        gt += toff
    ttot = gt

    # ---- per-core slot data -------------------------------------------------
    idx_all = np.zeros((NCORES, P, ttot * 8), dtype=np.int16)
    dlf_all = np.full((NCORES, ttot, P), -1.0, dtype=np.float32)
    dcf_all = np.zeros((NCORES, ttot, P), dtype=np.float32)
    for m in range(NCORES):
        dl, cl = core_dl[m], core_col[m]
        blk = dl >> 7
        ch = cl // CHUNK
        # slot position of each edge: tiles of its (blk,ch) group, CSR order
        gkey = blk * nchunk + ch
        gcnt = np.bincount(gkey, minlength=nblk * nchunk).reshape(nblk, nchunk)
        starts128 = np.zeros((nblk, nchunk), dtype=np.int64)
        for bb in range(nblk):
            for c in range(nchunk):
                if tcnt[bb, c]:
                    starts128[bb, c] = tpos[(bb, c)] * P
        grp_start = np.zeros(nblk * nchunk + 1, dtype=np.int64)
        np.cumsum(gcnt.ravel(), out=grp_start[1:])
        within = np.arange(len(dl), dtype=np.int64) - grp_start[gkey]
        slot = starts128[blk, ch] + within
        tno = slot >> 7
        pno = slot & 127
        lcol = (cl - ch * CHUNK).astype(np.int16)
        # wrapped idx layout: value for slot j of tile t lives at
        # [16 rows](j%16), col t*8 + j//16, replicated over 8 groups of 16
        flat = np.zeros((ttot, P), dtype=np.int16)
        flat[tno, pno] = lcol
        wrapped = flat.reshape(ttot, 8, 16).transpose(2, 0, 1).reshape(16, ttot * 8)
        idx_all[m] = np.tile(wrapped, (8, 1))
        dlf_all[m][tno, pno] = (dl & 127).astype(np.float32)
        dcf_all[m][tno, pno] = d32[cl]

    # per-tile destination window (32-aligned; union over cores)
    valid = dlf_all >= 0
    gmin = np.where(valid, dlf_all, 128.0).min(axis=(0, 2))
    gmax = np.where(valid, dlf_all, -1.0).max(axis=(0, 2))
    gmin = np.minimum(gmin, gmax.clip(0))  # empty tile -> [0, 0]
    lo32 = (gmin.astype(np.int64) // 32) * 32
    fits32 = (gmax < lo32 + 32) & (lo32 < 96)  # base partition 96 not encodable
    fits64a = gmax < 64
    fits64b = gmin >= 64
    win_w = np.where(fits32, 32, np.where(fits64a | fits64b, 64, 128)).astype(np.int64)
    win_lo = np.where(
        fits32, lo32, np.where(fits64a, 0, np.where(fits64b, 64, 0))
    ).astype(np.int64)
    dl_all = np.empty((NCORES, P, ttot), dtype=np.float32)
    dc_all = np.empty((NCORES, P, ttot), dtype=np.float32)
    for m in range(NCORES):
        rel = dlf_all[m] - win_lo[:, None]
        rel[~valid[m]] = -1.0
        dl_all[m] = rel.T
        dc_all[m] = dcf_all[m].T

    # ---- per-node inputs ----------------------------------------------------
    xf = np.asarray(x, dtype=np.float32)
    x_full = np.zeros((n_y, din), dtype=np.float16)
    x_full[:N] = xf
    xs_all = np.zeros((NCORES, npc_pad, din), dtype=np.float16)
    dv_all = np.ones((NCORES, P, nblk), dtype=np.float32)
    cb_all = np.zeros((NCORES, P, nblk), dtype=np.float32)
    for m in range(NCORES):
        xs_all[m, :npc] = xf[m * npc : (m + 1) * npc]
        dm = np.ones(npc_pad, dtype=np.float32)
        dm[:npc] = d32[m * npc : (m + 1) * npc]
        dv_all[m] = dm.reshape(nblk, P).T
        cm_ = np.zeros(npc_pad, dtype=np.float32)
        cm_[:npc] = cbv[m * npc : (m + 1) * npc]
        cb_all[m] = cm_.reshape(nblk, P).T

    meta = dict(
        N=N, din=din, dout=dout, npc=npc, nblk=nblk, npc_pad=npc_pad,
        nchunk=nchunk, n_y=n_y, ttot=ttot,
        sblocks=sblocks, sb_tiles=sb_tiles, sb_calls=sb_calls,
        blk_runs=blk_runs, tile_base=tile_base,
        win_lo=win_lo, win_w=win_w,
    )
    data = dict(
        idx_all=idx_all, dl_all=dl_all, dc_all=dc_all,
        x_full=x_full, xs_all=xs_all, dv_all=dv_all, cb_all=cb_all,
    )
    return meta, data


def kernel(x, edge_index, W, b):
    x = np.asarray(x, dtype=np.float32)
    W = np.asarray(W, dtype=np.float32)
    b = np.asarray(b, dtype=np.float32)
    edge_index = np.asarray(edge_index)
    meta, data = _prep(x, edge_index, W, b)
    N, din, dout = meta["N"], meta["din"], meta["dout"]

    key = (
        "l", N, din, dout,
        tuple(int(t) for t in np.asarray(meta["sb_tiles"])),
        meta["ttot"],
        tuple(int(v) for v in meta["win_lo"]),
        tuple(int(v) for v in meta["win_w"]),
    )
    if key not in _cache:
        _cache[key] = _build(meta)
    nc = _cache[key]

    wt = np.ascontiguousarray(W.T)
    brep = np.repeat(b[None, :], P, axis=0).astype(np.float32)
    in_maps = [
        {
            "x_t": data["x_full"],
            "xs_t": data["xs_all"][m],
            "idx_t": data["idx_all"][m],
            "dl_t": data["dl_all"][m],
            "dc_t": data["dc_all"][m],
            "dv_t": data["dv_all"][m],
            "cb_t": data["cb_all"][m],
            "wt_t": wt,
            "brep_t": brep,
        }
        for m in range(NCORES)
    ]
    res = run_bass_kernel_spmd(nc, in_maps, list(range(NCORES))).results

    LAST.clear()
    LAST.update(launches=[("launch", nc, in_maps)])

    out = np.empty((N, dout), dtype=np.float32)
    for m in range(NCORES):
        out[m * meta["npc"] : (m + 1) * meta["npc"]] = res[m]["out_t"][: meta["npc"]]
    return out
